# revision 44
# baseline (speedup 1.0000x reference)
"""HSE (hard squeeze-excite) Trainium2 Bass kernel.

Full inputs: x [32,56,56,256] f32, w1 [256,64], w2 [64,256].
out = x * hsigmoid(relu6(gap(x) @ w1) @ w2), gap = mean over H,W.

Sharding: pure data-parallel over batch, 4 samples per core on 8 cores.

Per-core layout (pair-granule pipeline): 3136 = 64*49, so one PAIR of
samples fills all 128 partitions: granule m holds sample 2m on
partitions 0-63 and sample 2m+1 on partitions 64-127, each partition
line holding 49 contiguous tokens. All bulk DMAs sit on ONE HWDGE ring
(sync engine) in program order: loads0, loads1, stores0, stores1 at
line rate.

Engine split (v1's bottleneck was DVE running both the GAP tree and the
gate multiply, ~54us serial):
 - GAP on the Tensor engine: psum[j,c] += maskT[p,j] * X[p,q,c]
   accumulated over the 49 token columns, float32r so the moving
   operand streams one pass. maskT carries 1/TOK so psum = mean.
 - The whole squeeze/excite chain on PE + ACT only (transpose trick
   for s->sT; relu6(v) = 6 - relu(6 - relu(v)) as ACT passes), so it
   never queues behind DVE multiplies. The excite and gate-replicate
   matmuls also run float32r (one pass instead of fp32's LOW+HIGH).
 - DVE does ONLY the broadcast gate multiplies.
Pipeline shaping:
 - Load/GAP chunks are three big pieces [16,16,17] (finer load chunking
   makes the DMA completion sems fire much later than the data lands,
   measured, and cascades into the gate chain).
 - Store/mult chunks are [4,12,16,17]: the tiny first piece means the
   first gated store is issued before the load stream drains the ring.
 - Only w1/w2 (real data, 132KB) ride the sync ring right behind the
   first x chunk (three DMAs with >=512B descriptors); the mask and
   ACT-bias constants are built by gpsimd memsets during the engine
   preamble, costing zero HBM ring bytes. (v1's five 256B-descriptor
   weight DMAs held 3 of the 8 DMA sem lanes until ~30us and stalled
   granule-1 loads behind them.)
"""

import numpy as np

B, H, W, C = 32, 56, 56, 256
CR = 64
NCORES = 8
BPC = B // NCORES            # 4 samples per core
TOK = H * W                  # 3136 tokens per sample
P = 128                      # SBUF partitions
NG = BPC // 2                # 2 granules (sample pairs) per core
TPL = TOK // 64              # 49 tokens per partition line
HP = 64                      # partitions per sample within a granule

# w1 lhsT halves ship as [128, 128] f32; w2 lhsT as [64, 256] f32r.
# All other constants (mask, replication, identity, ACT biases) are
# single-value position patterns built with gpsimd memsets during the
# engine-start preamble -- zero HBM ring bytes.
WPK = 128

_CACHE = {}


def _build():
    import concourse.bacc as bacc
    import concourse.tile as tile
    import concourse.mybir as mybir

    f32 = mybir.dt.float32
    f32r = mybir.dt.float32r
    op = mybir.AluOpType
    act = mybir.ActivationFunctionType

    nc = bacc.Bacc("TRN2", target_bir_lowering=False, debug=False)

    # x viewed per granule: [granule, half, 64 lines, 49 tokens, 256].
    # Declared float32r (same bits as f32) so the GAP matmuls stream one
    # pass; f32 views are bitcast where exact f32 semantics matter.
    x_d = nc.dram_tensor("x", [NG, 2, HP, TPL, C], f32r, kind="ExternalInput").ap()
    wpack_d = nc.dram_tensor("wpack", [P, WPK], f32, kind="ExternalInput").ap()
    cpack_d = nc.dram_tensor("cpack", [CR, 256], f32r, kind="ExternalInput").ap()
    rpack_d = nc.dram_tensor("rpack", [2, 130], f32r, kind="ExternalInput").ap()
    o_d = nc.dram_tensor("out", [NG, 2, HP, TPL, C], f32, kind="ExternalOutput").ap()

    # token chunks within a granule: loads + GAP use three big chunks
    # (finer load chunking makes the DMA completion sems fire much later
    # than the data and cascades, measured), stores + mults use a small
    # HEAD chunk so the first gated store is issued before the ring
    # drains the loads.
    CHUNKS = [(0, 16), (16, 32), (32, 49)]
    # store/mult chunks: tiny head so the first gated store is issued
    # before the ring drains the loads (best-measured layout; a tiny
    # tail chunk and a SWDGE store queue were both tried and regressed)
    SCHUNKS = [(0, 2), (2, 16), (16, 32), (32, 49)]

    with tile.TileContext(nc) as tc:
        with tc.tile_pool(name="big", bufs=1) as big, \
             tc.tile_pool(name="small", bufs=1) as small, \
             tc.tile_pool(name="se", bufs=2) as se, \
             tc.tile_pool(name="ypool", bufs=4) as ypool, \
             tc.tile_pool(name="gpool", bufs=2) as gpool, \
             tc.tile_pool(name="psg", bufs=2, space="PSUM") as psg, \
             tc.tile_pool(name="pss", bufs=1, space="PSUM") as pss, \
             tc.tile_pool(name="gps", bufs=2, space="PSUM") as gps:

            X = big.tile([P, NG, TPL, C], f32r)     # both granules, ~100KB/part
            wpack = small.tile([P, WPK], f32)
            cpack = small.tile([CR, 256], f32r)

            w1s = [wpack[:, 0:CR], wpack[:, CR : 2 * CR]]
            w2s = cpack[:]
            maskT = small.tile([P, 2], f32r)
            maskF = small.tile([P, 2], f32)
            rpack = small.tile([2, 130], f32r)
            rt2 = rpack[0:2, 0:128]
            id2 = rpack[0:2, 128:130].bitcast(f32)
            six = small.tile([P, 1], f32)
            three = small.tile([P, 1], f32)

            # single-value pattern constants via memset (no DMA bytes);
            # all complete during the preamble, long before first use.
            # (memset APs must start at a gpsimd core boundary and only
            # support plain dtypes, so the row-1 patterns rt2/id2 ship in
            # a tiny 1KB DMA, and maskT stages through an f32 tile with an
            # ACT copy as the f32r "rounding" producer.)
            nc.gpsimd.memset(maskF[0:HP, 0:1], 1.0 / TOK)
            nc.gpsimd.memset(maskF[0:HP, 1:2], 0.0)
            nc.gpsimd.memset(maskF[HP:P, 0:1], 0.0)
            nc.gpsimd.memset(maskF[HP:P, 1:2], 1.0 / TOK)
            nc.scalar.copy(maskT[:], maskF[:])
            nc.gpsimd.memset(six[:], 6.0)
            nc.gpsimd.memset(three[:], 3.0)

            # ---- loads: granule 0 then granule 1, chunked, one ring.
            # The constant loads ride the same ring right behind the first
            # x chunk: the bulk stream gets first-byte priority and the
            # constants still land long before the squeeze needs them. ----
            first = True
            for m in range(NG):
                for (t0, t1) in CHUNKS:
                    nc.sync.dma_start(
                        X[:, m, t0:t1, :], x_d[m, :, :, t0:t1, :]
                    )
                    if first:
                        nc.sync.dma_start(wpack[:], wpack_d[:])
                        nc.sync.dma_start(cpack[:], cpack_d[:])
                        nc.sync.dma_start(rpack[:], rpack_d[:])
                        first = False

            G_sb = [None] * NG
            for m in range(NG):
                # ---- GAP on PE: mean over all 3136 tokens per sample ----
                # s_ps[j, c] = sum_q sum_p maskT[p, j] * X[p, q, c]
                s_ps = psg.tile([2, C], f32, tag="s")
                for (t0, t1) in CHUNKS:
                    for q in range(t0, t1):
                        nc.tensor.matmul(
                            s_ps[:],
                            maskT,
                            X[:, m, q, :],
                            start=(q == 0),
                            stop=(q == TPL - 1),
                        )
                s_sb = se.tile([2, C], f32, tag="ssb")
                nc.scalar.copy(s_sb[:], s_ps[:])

                # ---- transpose s -> sT halves (PE), c onto partitions ----
                sT_sb = se.tile([P, 2, 2], f32, tag="sTsb")
                for h in range(2):
                    sT_ps = pss.tile([P, 2], f32, tag=f"sT{h}")
                    nc.tensor.transpose(
                        sT_ps[:], s_sb[:, 128 * h : 128 * (h + 1)], id2
                    )
                    nc.scalar.copy(sT_sb[:, h, :], sT_ps[:])

                # ---- squeeze: zT[r, j] = sum_c w1[c, r] * s[c, j] ----
                zT_ps = pss.tile([CR, 2], f32, tag="zT")
                nc.tensor.matmul(zT_ps[:], w1s[0], sT_sb[:, 0, :], start=True, stop=False)
                nc.tensor.matmul(zT_ps[:], w1s[1], sT_sb[:, 1, :], start=False, stop=True)

                # relu6(z) = 6 - relu(6 - relu(z)), all on ACT; the final
                # affine lands in an f32r tile so the excite matmul can run
                # single-pass f32r
                z_ab = se.tile([CR, 2, 2], f32, tag="zab")
                z_r = se.tile([CR, 2], f32r, tag="zr")
                nc.scalar.activation(z_ab[:, 0, :], zT_ps[:], act.Relu)
                nc.scalar.activation(z_ab[:, 1, :], z_ab[:, 0, :], act.Relu, bias=six[0:CR, :], scale=-1.0)
                nc.scalar.activation(z_r[:], z_ab[:, 1, :], act.Copy, bias=6.0, scale=-1.0)

                # ---- excite: y[j, c] = sum_r relu6(z)[r, j] * w2[r, c] ----
                y_ps = pss.tile([2, C], f32, tag="y")
                nc.tensor.matmul(y_ps[:], z_r[:], w2s, start=True, stop=True)

                # hsigmoid(y) = (6 - relu(6 - relu(y + 3))) / 6, on ACT
                g_ab = se.tile([2, 2, C], f32, tag="gab")
                g_r = se.tile([2, C], f32r, tag="gr")
                nc.scalar.activation(g_ab[:, 0, :], y_ps[:], act.Relu, bias=three[0:2, :])
                nc.scalar.activation(g_ab[:, 1, :], g_ab[:, 0, :], act.Relu, bias=six[0:2, :], scale=-1.0)
                nc.scalar.activation(g_r[:], g_ab[:, 1, :], act.Copy, bias=1.0, scale=-1.0 / 6.0)

                # replicate gate rows onto lines: G[p, c] = g[p // HP, c]
                G_ps = gps.tile([P, C], f32, tag="G")
                nc.tensor.matmul(G_ps[:], rt2, g_r[:], start=True, stop=True)
                G_sb[m] = gpool.tile([P, C], f32, tag="Gsb", name=f"G_sb{m}")
                nc.scalar.copy(G_sb[m][:], G_ps[:])

                # ---- gate multiply (DVE) into rotating chunk buffers,
                # stores on the same ring; X stays read-only (the BIR
                # verifier treats any write to X as feeding the f32r
                # matmuls regardless of program order) ----
                for (t0, t1) in SCHUNKS:
                    n = t1 - t0
                    yc = ypool.tile([P, 17, C], f32, tag="y")
                    gb = G_sb[m][:].unsqueeze(1).broadcast_to([P, n, C])
                    nc.vector.tensor_tensor(
                        yc[:, 0:n, :], X[:, m, t0:t1, :].bitcast(f32), gb, op=op.mult
                    )
                    nc.sync.dma_start(o_d[m, :, :, t0:t1, :], yc[:, 0:n, :])

    nc.compile()
    return nc


def _make_packs(w1, w2):
    # w1 lhsT halves: w1s[j][p, r] = w1[j*128 + p, r]
    wpack = np.ascontiguousarray(
        w1.reshape(2, 128, CR).transpose(1, 0, 2).reshape(128, 128)
    )
    cpack = np.ascontiguousarray(w2)
    rpack = np.zeros((2, 130), dtype=np.float32)
    for j in range(2):
        rpack[j, HP * j : HP * (j + 1)] = 1.0
        rpack[j, 128 + j] = 1.0
    return wpack, cpack, rpack


def _in_maps(x, w1, w2):
    x = np.ascontiguousarray(x, dtype=np.float32)
    w1 = np.ascontiguousarray(w1, dtype=np.float32)
    w2 = np.ascontiguousarray(w2, dtype=np.float32)
    wpack, cpack, rpack = _make_packs(w1, w2)

    in_maps = []
    for c in range(NCORES):
        # [4 samples, 3136 tok, C] -> [NG, 2, HP, TPL, C]
        shard = x[c * BPC : (c + 1) * BPC].reshape(NG, 2, HP, TPL, C)
        in_maps.append({"x": shard, "wpack": wpack, "cpack": cpack, "rpack": rpack})
    return in_maps


def kernel(x, w1, w2):
    from concourse.bass_utils import run_bass_kernel_spmd

    if "nc" not in _CACHE:
        _CACHE["nc"] = _build()
    nc = _CACHE["nc"]

    res = run_bass_kernel_spmd(nc, _in_maps(x, w1, w2), core_ids=list(range(NCORES)))
    out = np.empty((B, H, W, C), dtype=np.float32)
    for c in range(NCORES):
        out[c * BPC : (c + 1) * BPC] = res.results[c]["out"].reshape(BPC, H, W, C)
    return out
